# revision 25
# baseline (speedup 1.0000x reference)
"""ArcFace-style AngularPenaltySMLoss on 8 TRN2 NeuronCores.

Reference computation (N=4096, IN_F=512, OUT_F=20000, S=64, M=0.5):
    xn   = x / max(||x||_row, 1e-12)
    wf   = xn @ W.T                         # [N, OUT_F]
    diag = wf[i, labels[i]]
    num  = S*cos(acos(clip(diag)) + M)
    excl = sum_j exp(S*wf[i,j]) - exp(S*diag)
    L    = num - log(exp(num) + excl)
    out  = (-mean(L), wf)

Sharding: column-parallel over OUT_F (2500 cols/core, zero-padded to 2560).
Each core computes its [4096, 2500] slice of wf, the partial row sum-exp
over its columns, and the diag dot-products for its 512-row slice
(diag_i = xn_i . W[labels_i] is row-parallel).  Host does the O(N) scalar
epilogue and concatenates wf slices.

Device dataflow per core:
  - transposing DMA loads put the contraction dim (IN_F) on partitions:
      xT[k]  [128, 4096]  <- xn_bf16[:, 128k:128k+128].T
      wT[k]  [128, 2560]  <- w_bf16[:, 128k:128k+128].T
  - per (col-tile c, row-tile r): 4 accumulating bf16 matmuls -> PSUM f32
  - ScalarE: exp(S * psum) with fused accum_out -> partial row sum-exp
  - VectorE: copy psum -> SBUF f32, DMA out to wf
  - diag: tensor_tensor_reduce(xn_rows * wlab) row-sums
"""

import numpy as np
import ml_dtypes

from concourse import bass, bacc, tile, mybir

N, IN_F, OUT_F = 4096, 512, 20000
S, M, EPS = 64.0, 0.5, 1e-7
NCORES = 8
COLS = OUT_F // NCORES          # 2500 columns per core
ROWS_PC = N // NCORES           # 512 diag rows per core
RT = N // 128                   # 32 row tiles
KT = IN_F // 128                # 4 contraction tiles

_BF16 = mybir.dt.bfloat16
_F32 = mybir.dt.float32

_cached = {}


def _build_nc():
    nc = bacc.Bacc("TRN2", target_bir_lowering=False)

    # host-pre-transposed operands: contraction dim (IN_F) leading
    xnT_d = nc.declare_dram_parameter("xnT", [IN_F, N], _BF16, isOutput=False)
    wT_d = nc.declare_dram_parameter("wT", [IN_F, COLS], _BF16, isOutput=False)
    xnr_d = nc.declare_dram_parameter("xnr", [ROWS_PC, IN_F], _F32, isOutput=False)
    wlab_d = nc.declare_dram_parameter("wlab", [ROWS_PC, IN_F], _F32, isOutput=False)
    wf_d = nc.declare_dram_parameter("wf", [N, COLS], _BF16, isOutput=True)
    # 3 exp-sum groups per row tile, summed on host
    se_d = nc.declare_dram_parameter("se", [128, RT * 3], _F32, isOutput=True)
    dg_d = nc.declare_dram_parameter("dg", [128, ROWS_PC // 128], _F32, isOutput=True)

    with tile.TileContext(nc) as tc:
        with (
            tc.tile_pool(name="big", bufs=1) as big,
            tc.tile_pool(name="work", bufs=4) as work,
            tc.tile_pool(name="scr", bufs=2) as scr,
            tc.tile_pool(name="acc", bufs=1) as accp,
            tc.tile_pool(name="psw", bufs=3, space="PSUM") as psw,
            tc.tile_pool(name="psn", bufs=2, space="PSUM") as psn,
        ):
            # --- PE pre-warm: dummy matmuls while operands stream in -------
            # (HAM releases the PE clock gate after ~3.4us of sustained work)
            dumt = scr.tile([128, 512], _BF16, tag="dumt", bufs=1)
            nc.vector.memset(dumt[:], 0.0)
            dump = psn.tile([128, 512], _F32, tag="ptn", name="dummy_psum")
            for i in range(12):
                nc.tensor.matmul(
                    dump[:], dumt[:, 0:128], dumt[:], start=True, stop=True
                )

            # --- operand loads (host already transposed); chunked so the
            # first row tiles' matmuls can start as soon as possible -------
            XCH = 4          # xT row chunks per k-tile
            XCW = N // XCH   # 1024 rows per chunk
            GRP = [(0, 1024, 1024), (1024, 1024, 1024), (2048, 452, 452)]
            xT = [
                [
                    big.tile([128, XCW], _BF16, tag=f"xt{k}_{j}", name=f"xt{k}_{j}")
                    for j in range(XCH)
                ]
                for k in range(KT)
            ]
            wT = [
                [
                    big.tile([128, gw], _BF16, tag=f"wt{k}_{gi}", name=f"wt{k}_{gi}")
                    for gi, (goff, gw, gv) in enumerate(GRP)
                ]
                for k in range(KT)
            ]
            for j in range(XCH):
                for k in range(KT):
                    nc.sync.dma_start(
                        out=xT[k][j][:],
                        in_=xnT_d[k * 128:(k + 1) * 128, j * XCW:(j + 1) * XCW],
                    )
                if j < len(GRP):
                    gi, (goff, gw, gv) = j, GRP[j]
                    for k in range(KT):
                        nc.sync.dma_start(
                            out=wT[k][gi][:],
                            in_=wT_d[k * 128:(k + 1) * 128, goff:goff + gw],
                        )

            # --- diag: rowsum(xn_rows * wlab), early (DVE idle in prologue)
            dgt = accp.tile([128, ROWS_PC // 128], _F32)
            for t in range(ROWS_PC // 128):
                xnt = scr.tile([128, IN_F], _F32, tag="xnt", name=f"xnt{t}")
                wlt = scr.tile([128, IN_F], _F32, tag="wlt", name=f"wlt{t}")
                prod = scr.tile([128, IN_F], _F32, tag="prod", name=f"prod{t}")
                nc.sync.dma_start(out=xnt[:], in_=xnr_d[t * 128:(t + 1) * 128, :])
                nc.sync.dma_start(out=wlt[:], in_=wlab_d[t * 128:(t + 1) * 128, :])
                nc.vector.tensor_mul(prod[:], xnt[:], wlt[:])
                nc.vector.tensor_reduce(
                    dgt[:, t:t + 1],
                    prod[:],
                    axis=mybir.AxisListType.X,
                    op=mybir.AluOpType.add,
                )
            nc.sync.dma_start(out=dg_d[:], in_=dgt[:])

            # --- accumulator for per-group exp sums ------------------------
            se_acc = accp.tile([128, RT * 3], _F32)

            # --- main loop: one row tile at a time -------------------------
            # groups of PSUM banks: [2 banks, 2 banks, 1 bank] = 2560 cols
            for r in range(RT):
                xch, xoff = r // (RT // XCH), (r % (RT // XCH)) * 128
                ot = work.tile([128, COLS], _BF16, tag="ot")
                for gi, (goff, gw, gvalid) in enumerate(GRP):
                    pool = psw if gw == 1024 else psn
                    pt = pool.tile(
                        [128, gw], _F32, tag=f"ptw" if gw == 1024 else "ptn",
                        name=f"pt{gi}_{r}",
                    )
                    for k in range(KT):
                        for c0 in range(0, gw, 512):
                            cw = min(512, gw - c0)
                            nc.tensor.matmul(
                                pt[:, c0:c0 + cw],
                                xT[k][xch][:, xoff:xoff + 128],
                                wT[k][gi][:, c0:c0 + cw],
                                start=(k == 0),
                                stop=(k == KT - 1),
                            )
                    # exp(S*wf) with fused row-sum written straight to se_acc
                    et = scr.tile([128, 1024], _F32, tag="et", name=f"et{r}_{gi}")
                    nc.scalar.activation(
                        et[:, :gvalid],
                        pt[:, :gvalid],
                        mybir.ActivationFunctionType.Exp,
                        scale=S,
                        accum_out=se_acc[:, r * 3 + gi:r * 3 + gi + 1],
                    )
                    # evacuate wf group: PSUM f32 -> SBUF bf16
                    nc.vector.tensor_copy(
                        ot[:, goff:goff + gvalid], pt[:, :gvalid]
                    )
                    if gi == 1:
                        nc.sync.dma_start(
                            out=wf_d[r * 128:(r + 1) * 128, 0:2048],
                            in_=ot[:, 0:2048],
                        )
                nc.sync.dma_start(
                    out=wf_d[r * 128:(r + 1) * 128, 2048:COLS],
                    in_=ot[:, 2048:COLS],
                )

            nc.sync.dma_start(out=se_d[:], in_=se_acc[:])

    nc.compile()
    return nc


def _make_in_maps(x, labels, W):
    """Host prologue: exact f32 normalization (matches reference), bf16
    casts for the matmul operands, W[labels] row gather, per-core shards."""
    x = np.asarray(x, dtype=np.float32)
    W = np.asarray(W, dtype=np.float32)
    labels = np.asarray(labels).astype(np.int64)

    norm = np.maximum(
        np.sqrt(np.einsum("ij,ij->i", x, x, dtype=np.float32)), np.float32(1e-12)
    )
    xn32 = x / norm[:, None].astype(np.float32)
    xn_bf = xn32.astype(ml_dtypes.bfloat16)
    xnT_bf = np.ascontiguousarray(xn_bf.T)  # [IN_F, N]
    W_bf = W.astype(ml_dtypes.bfloat16)
    Wlab = np.ascontiguousarray(W[labels])  # [N, IN_F] f32 row gather

    in_maps = []
    for c in range(NCORES):
        wshT = np.ascontiguousarray(W_bf[c * COLS:(c + 1) * COLS].T)
        in_maps.append(
            {
                "xnT": xnT_bf,
                "wT": wshT,
                "xnr": np.ascontiguousarray(xn32[c * ROWS_PC:(c + 1) * ROWS_PC]),
                "wlab": np.ascontiguousarray(Wlab[c * ROWS_PC:(c + 1) * ROWS_PC]),
            }
        )
    return in_maps


class _FastSpmd:
    """Cached-jit SPMD executor (mirrors bass2jax.run_bass_via_pjrt, but
    builds the jitted shard_map executable once instead of per call)."""

    def __init__(self, nc, n_cores):
        import jax
        from jax.sharding import Mesh, PartitionSpec
        from jax.experimental.shard_map import shard_map
        from concourse import bass2jax

        bass2jax.install_neuronx_cc_hook()
        assert nc.dbg_addr is None
        self.n_cores = n_cores
        partition_name = (
            nc.partition_id_tensor.name if nc.partition_id_tensor else None
        )
        in_names, out_names, out_avals, zero_outs = [], [], [], []
        for alloc in nc.m.functions[0].allocations:
            if not isinstance(alloc, mybir.MemoryLocationSet):
                continue
            name = alloc.memorylocations[0].name
            if alloc.kind == "ExternalInput":
                if name != partition_name:
                    in_names.append(name)
            elif alloc.kind == "ExternalOutput":
                shape = tuple(alloc.tensor_shape)
                dtype = mybir.dt.np(alloc.dtype)
                out_names.append(name)
                out_avals.append(jax.core.ShapedArray(shape, dtype))
                zero_outs.append(np.zeros(shape, dtype))
        self.n_params = len(in_names)
        self.out_names = out_names
        self.out_avals = out_avals
        self.zero_outs = zero_outs
        self.in_param_names = list(in_names)
        all_in_names = in_names + out_names
        if partition_name is not None:
            all_in_names.append(partition_name)
        n_outs = len(out_avals)
        donate = tuple(range(self.n_params, self.n_params + n_outs))

        def _body(*args):
            operands = list(args)
            if partition_name is not None:
                operands.append(bass2jax.partition_id_tensor())
            outs = bass2jax._bass_exec_p.bind(
                *operands,
                out_avals=tuple(out_avals),
                in_names=tuple(all_in_names),
                out_names=tuple(out_names),
                lowering_input_output_aliases=(),
                sim_require_finite=True,
                sim_require_nnan=True,
                nc=nc,
            )
            return tuple(outs)

        devices = jax.devices()[:n_cores]
        assert len(devices) == n_cores
        mesh = Mesh(np.asarray(devices), ("core",))
        in_specs = (PartitionSpec("core"),) * (self.n_params + n_outs)
        out_specs = (PartitionSpec("core"),) * n_outs
        self._fn = jax.jit(
            shard_map(
                _body, mesh=mesh, in_specs=in_specs, out_specs=out_specs,
                check_rep=False,
            ),
            donate_argnums=donate,
            keep_unused=True,
        )

    def __call__(self, in_maps):
        n = self.n_cores
        concat_in = [
            np.concatenate([np.asarray(in_maps[c][k]) for c in range(n)], axis=0)
            for k in self.in_param_names
        ]
        concat_zeros = [
            np.zeros((n * z.shape[0], *z.shape[1:]), z.dtype)
            for z in self.zero_outs
        ]
        out_arrs = self._fn(*concat_in, *concat_zeros)
        return [
            {
                name: np.asarray(out_arrs[i]).reshape(
                    n, *self.out_avals[i].shape
                )[c]
                for i, name in enumerate(self.out_names)
            }
            for c in range(n)
        ]


def kernel(x, labels, W):
    if "exec" not in _cached:
        _cached["nc"] = _build_nc()
        _cached["exec"] = _FastSpmd(_cached["nc"], NCORES)

    in_maps = _make_in_maps(x, labels, W)
    res = _cached["exec"](in_maps)

    # host epilogue: gather/unshard + O(N) scalar tail
    wf = np.empty((N, OUT_F), dtype=np.float32)
    se = np.zeros(N, dtype=np.float64)
    dg = np.empty(N, dtype=np.float32)
    for c in range(NCORES):
        wf[:, c * COLS:(c + 1) * COLS] = res[c]["wf"]  # bf16 -> f32 cast
        se_c = res[c]["se"].astype(np.float64).reshape(128, RT, 3).sum(-1)
        se += se_c.T.reshape(-1)
        dg[c * ROWS_PC:(c + 1) * ROWS_PC] = res[c]["dg"].T.reshape(-1)

    d64 = dg.astype(np.float64)
    dc = np.clip(d64, -1.0 + EPS, 1.0 - EPS)
    numerator = S * (dc * np.cos(M) - np.sqrt(1.0 - dc * dc) * np.sin(M))
    excl = se - np.exp(S * d64)
    L = numerator - np.log(np.exp(numerator) + excl)
    loss = np.array(-np.mean(L), dtype=np.float32)
    return (loss, wf)


# revision 26
# speedup vs baseline: 1.2203x; 1.2203x over previous
"""ArcFace-style AngularPenaltySMLoss on 8 TRN2 NeuronCores.

Reference computation (N=4096, IN_F=512, OUT_F=20000, S=64, M=0.5):
    xn   = x / max(||x||_row, 1e-12)
    wf   = xn @ W.T                         # [N, OUT_F]
    diag = wf[i, labels[i]]
    num  = S*cos(acos(clip(diag)) + M)
    excl = sum_j exp(S*wf[i,j]) - exp(S*diag)
    L    = num - log(exp(num) + excl)
    out  = (-mean(L), wf)

Sharding: column-parallel over OUT_F (2500 cols/core, zero-padded to 2560).
Each core computes its [4096, 2500] slice of wf, the partial row sum-exp
over its columns, and the diag dot-products for its 512-row slice
(diag_i = xn_i . W[labels_i] is row-parallel).  Host does the O(N) scalar
epilogue and concatenates wf slices.

Device dataflow per core:
  - transposing DMA loads put the contraction dim (IN_F) on partitions:
      xT[k]  [128, 4096]  <- xn_bf16[:, 128k:128k+128].T
      wT[k]  [128, 2560]  <- w_bf16[:, 128k:128k+128].T
  - per (col-tile c, row-tile r): 4 accumulating bf16 matmuls -> PSUM f32
  - ScalarE: exp(S * psum) with fused accum_out -> partial row sum-exp
  - VectorE: copy psum -> SBUF f32, DMA out to wf
  - diag: tensor_tensor_reduce(xn_rows * wlab) row-sums
"""

import numpy as np
import ml_dtypes

from concourse import bass, bacc, tile, mybir

N, IN_F, OUT_F = 4096, 512, 20000
S, M, EPS = 64.0, 0.5, 1e-7
NCORES = 8
COLS = OUT_F // NCORES          # 2500 columns per core
ROWS_PC = N // NCORES           # 512 diag rows per core
RT = N // 128                   # 32 row tiles
KT = IN_F // 128                # 4 contraction tiles

_BF16 = mybir.dt.bfloat16
_F32 = mybir.dt.float32

_cached = {}


def _build_nc():
    nc = bacc.Bacc("TRN2", target_bir_lowering=False)

    # host-pre-transposed operands: contraction dim (IN_F) leading
    xnT_d = nc.declare_dram_parameter("xnT", [IN_F, N], _BF16, isOutput=False)
    wT_d = nc.declare_dram_parameter("wT", [IN_F, COLS], _BF16, isOutput=False)
    xnr_d = nc.declare_dram_parameter("xnr", [ROWS_PC, IN_F], _F32, isOutput=False)
    wlab_d = nc.declare_dram_parameter("wlab", [ROWS_PC, IN_F], _F32, isOutput=False)
    wf_d = nc.declare_dram_parameter("wf", [N, COLS], _BF16, isOutput=True)
    # 3 exp-sum groups per row tile, summed on host
    se_d = nc.declare_dram_parameter("se", [128, RT * 3], _F32, isOutput=True)
    dg_d = nc.declare_dram_parameter("dg", [128, ROWS_PC // 128], _F32, isOutput=True)

    with tile.TileContext(nc) as tc:
        with (
            tc.tile_pool(name="big", bufs=1) as big,
            tc.tile_pool(name="work", bufs=4) as work,
            tc.tile_pool(name="scr", bufs=2) as scr,
            tc.tile_pool(name="acc", bufs=1) as accp,
            tc.tile_pool(name="psw", bufs=3, space="PSUM") as psw,
            tc.tile_pool(name="psn", bufs=2, space="PSUM") as psn,
        ):
            # --- PE pre-warm: dummy matmuls while operands stream in -------
            # (HAM releases the PE clock gate after ~3.4us of sustained work)
            dumt = scr.tile([128, 512], _BF16, tag="dumt", bufs=1)
            nc.vector.memset(dumt[:], 0.0)
            dump = psn.tile([128, 512], _F32, tag="ptn", name="dummy_psum")
            for i in range(12):
                nc.tensor.matmul(
                    dump[:], dumt[:, 0:128], dumt[:], start=True, stop=True
                )

            # --- operand loads (host already transposed); chunked so the
            # first row tiles' matmuls can start as soon as possible -------
            XCH = 4          # xT row chunks per k-tile
            XCW = N // XCH   # 1024 rows per chunk
            GRP = [(0, 1024, 1024), (1024, 1024, 1024), (2048, 452, 452)]
            xT = [
                [
                    big.tile([128, XCW], _BF16, tag=f"xt{k}_{j}", name=f"xt{k}_{j}")
                    for j in range(XCH)
                ]
                for k in range(KT)
            ]
            wT = [
                [
                    big.tile([128, gw], _BF16, tag=f"wt{k}_{gi}", name=f"wt{k}_{gi}")
                    for gi, (goff, gw, gv) in enumerate(GRP)
                ]
                for k in range(KT)
            ]
            # issue order: everything row-tile 0 needs first (x chunk 0 +
            # all three W groups), then the remaining x chunks
            for k in range(KT):
                nc.sync.dma_start(
                    out=xT[k][0][:], in_=xnT_d[k * 128:(k + 1) * 128, 0:XCW]
                )
            for gi, (goff, gw, gv) in enumerate(GRP):
                for k in range(KT):
                    nc.sync.dma_start(
                        out=wT[k][gi][:],
                        in_=wT_d[k * 128:(k + 1) * 128, goff:goff + gw],
                    )
            for j in range(1, XCH):
                for k in range(KT):
                    nc.sync.dma_start(
                        out=xT[k][j][:],
                        in_=xnT_d[k * 128:(k + 1) * 128, j * XCW:(j + 1) * XCW],
                    )

            # --- diag: rowsum(xn_rows * wlab), early (DVE idle in prologue)
            dgt = accp.tile([128, ROWS_PC // 128], _F32)
            for t in range(ROWS_PC // 128):
                xnt = scr.tile([128, IN_F], _F32, tag="xnt", name=f"xnt{t}")
                wlt = scr.tile([128, IN_F], _F32, tag="wlt", name=f"wlt{t}")
                prod = scr.tile([128, IN_F], _F32, tag="prod", name=f"prod{t}")
                nc.sync.dma_start(out=xnt[:], in_=xnr_d[t * 128:(t + 1) * 128, :])
                nc.sync.dma_start(out=wlt[:], in_=wlab_d[t * 128:(t + 1) * 128, :])
                nc.vector.tensor_mul(prod[:], xnt[:], wlt[:])
                nc.vector.tensor_reduce(
                    dgt[:, t:t + 1],
                    prod[:],
                    axis=mybir.AxisListType.X,
                    op=mybir.AluOpType.add,
                )
            nc.sync.dma_start(out=dg_d[:], in_=dgt[:])

            # --- accumulator for per-group exp sums ------------------------
            se_acc = accp.tile([128, RT * 3], _F32)

            # --- main loop: one row tile at a time -------------------------
            # groups of PSUM banks: [2 banks, 2 banks, 1 bank] = 2560 cols
            for r in range(RT):
                xch, xoff = r // (RT // XCH), (r % (RT // XCH)) * 128
                ot = work.tile([128, COLS], _BF16, tag="ot")
                for gi, (goff, gw, gvalid) in enumerate(GRP):
                    pool = psw if gw == 1024 else psn
                    pt = pool.tile(
                        [128, gw], _F32, tag=f"ptw" if gw == 1024 else "ptn",
                        name=f"pt{gi}_{r}",
                    )
                    for k in range(KT):
                        for c0 in range(0, gw, 512):
                            cw = min(512, gw - c0)
                            nc.tensor.matmul(
                                pt[:, c0:c0 + cw],
                                xT[k][xch][:, xoff:xoff + 128],
                                wT[k][gi][:, c0:c0 + cw],
                                start=(k == 0),
                                stop=(k == KT - 1),
                            )
                    # exp(S*wf) with fused row-sum written straight to se_acc
                    et = scr.tile([128, 1024], _F32, tag="et", name=f"et{r}_{gi}")
                    nc.scalar.activation(
                        et[:, :gvalid],
                        pt[:, :gvalid],
                        mybir.ActivationFunctionType.Exp,
                        scale=S,
                        accum_out=se_acc[:, r * 3 + gi:r * 3 + gi + 1],
                    )
                    # evacuate wf group: PSUM f32 -> SBUF bf16
                    nc.vector.tensor_copy(
                        ot[:, goff:goff + gvalid], pt[:, :gvalid]
                    )
                    if gi == 1:
                        nc.sync.dma_start(
                            out=wf_d[r * 128:(r + 1) * 128, 0:2048],
                            in_=ot[:, 0:2048],
                        )
                nc.sync.dma_start(
                    out=wf_d[r * 128:(r + 1) * 128, 2048:COLS],
                    in_=ot[:, 2048:COLS],
                )

            nc.sync.dma_start(out=se_d[:], in_=se_acc[:])

    nc.compile()
    return nc


def _make_in_maps(x, labels, W):
    """Host prologue: exact f32 normalization (matches reference), bf16
    casts for the matmul operands, W[labels] row gather, per-core shards."""
    x = np.asarray(x, dtype=np.float32)
    W = np.asarray(W, dtype=np.float32)
    labels = np.asarray(labels).astype(np.int64)

    norm = np.maximum(
        np.sqrt(np.einsum("ij,ij->i", x, x, dtype=np.float32)), np.float32(1e-12)
    )
    xn32 = x / norm[:, None].astype(np.float32)
    xn_bf = xn32.astype(ml_dtypes.bfloat16)
    xnT_bf = np.ascontiguousarray(xn_bf.T)  # [IN_F, N]
    W_bf = W.astype(ml_dtypes.bfloat16)
    Wlab = np.ascontiguousarray(W[labels])  # [N, IN_F] f32 row gather

    in_maps = []
    for c in range(NCORES):
        wshT = np.ascontiguousarray(W_bf[c * COLS:(c + 1) * COLS].T)
        in_maps.append(
            {
                "xnT": xnT_bf,
                "wT": wshT,
                "xnr": np.ascontiguousarray(xn32[c * ROWS_PC:(c + 1) * ROWS_PC]),
                "wlab": np.ascontiguousarray(Wlab[c * ROWS_PC:(c + 1) * ROWS_PC]),
            }
        )
    return in_maps


class _FastSpmd:
    """Cached-jit SPMD executor (mirrors bass2jax.run_bass_via_pjrt, but
    builds the jitted shard_map executable once instead of per call)."""

    def __init__(self, nc, n_cores):
        import jax
        from jax.sharding import Mesh, PartitionSpec
        from jax.experimental.shard_map import shard_map
        from concourse import bass2jax

        bass2jax.install_neuronx_cc_hook()
        assert nc.dbg_addr is None
        self.n_cores = n_cores
        partition_name = (
            nc.partition_id_tensor.name if nc.partition_id_tensor else None
        )
        in_names, out_names, out_avals, zero_outs = [], [], [], []
        for alloc in nc.m.functions[0].allocations:
            if not isinstance(alloc, mybir.MemoryLocationSet):
                continue
            name = alloc.memorylocations[0].name
            if alloc.kind == "ExternalInput":
                if name != partition_name:
                    in_names.append(name)
            elif alloc.kind == "ExternalOutput":
                shape = tuple(alloc.tensor_shape)
                dtype = mybir.dt.np(alloc.dtype)
                out_names.append(name)
                out_avals.append(jax.core.ShapedArray(shape, dtype))
                zero_outs.append(np.zeros(shape, dtype))
        self.n_params = len(in_names)
        self.out_names = out_names
        self.out_avals = out_avals
        self.zero_outs = zero_outs
        self.in_param_names = list(in_names)
        all_in_names = in_names + out_names
        if partition_name is not None:
            all_in_names.append(partition_name)
        n_outs = len(out_avals)
        donate = tuple(range(self.n_params, self.n_params + n_outs))

        def _body(*args):
            operands = list(args)
            if partition_name is not None:
                operands.append(bass2jax.partition_id_tensor())
            outs = bass2jax._bass_exec_p.bind(
                *operands,
                out_avals=tuple(out_avals),
                in_names=tuple(all_in_names),
                out_names=tuple(out_names),
                lowering_input_output_aliases=(),
                sim_require_finite=True,
                sim_require_nnan=True,
                nc=nc,
            )
            return tuple(outs)

        devices = jax.devices()[:n_cores]
        assert len(devices) == n_cores
        mesh = Mesh(np.asarray(devices), ("core",))
        in_specs = (PartitionSpec("core"),) * (self.n_params + n_outs)
        out_specs = (PartitionSpec("core"),) * n_outs
        self._fn = jax.jit(
            shard_map(
                _body, mesh=mesh, in_specs=in_specs, out_specs=out_specs,
                check_rep=False,
            ),
            donate_argnums=donate,
            keep_unused=True,
        )

    def __call__(self, in_maps):
        n = self.n_cores
        concat_in = [
            np.concatenate([np.asarray(in_maps[c][k]) for c in range(n)], axis=0)
            for k in self.in_param_names
        ]
        concat_zeros = [
            np.zeros((n * z.shape[0], *z.shape[1:]), z.dtype)
            for z in self.zero_outs
        ]
        out_arrs = self._fn(*concat_in, *concat_zeros)
        return [
            {
                name: np.asarray(out_arrs[i]).reshape(
                    n, *self.out_avals[i].shape
                )[c]
                for i, name in enumerate(self.out_names)
            }
            for c in range(n)
        ]


def kernel(x, labels, W):
    if "exec" not in _cached:
        _cached["nc"] = _build_nc()
        _cached["exec"] = _FastSpmd(_cached["nc"], NCORES)

    in_maps = _make_in_maps(x, labels, W)
    res = _cached["exec"](in_maps)

    # host epilogue: gather/unshard + O(N) scalar tail
    wf = np.empty((N, OUT_F), dtype=np.float32)
    se = np.zeros(N, dtype=np.float64)
    dg = np.empty(N, dtype=np.float32)
    for c in range(NCORES):
        wf[:, c * COLS:(c + 1) * COLS] = res[c]["wf"]  # bf16 -> f32 cast
        se_c = res[c]["se"].astype(np.float64).reshape(128, RT, 3).sum(-1)
        se += se_c.T.reshape(-1)
        dg[c * ROWS_PC:(c + 1) * ROWS_PC] = res[c]["dg"].T.reshape(-1)

    d64 = dg.astype(np.float64)
    dc = np.clip(d64, -1.0 + EPS, 1.0 - EPS)
    numerator = S * (dc * np.cos(M) - np.sqrt(1.0 - dc * dc) * np.sin(M))
    excl = se - np.exp(S * d64)
    L = numerator - np.log(np.exp(numerator) + excl)
    loss = np.array(-np.mean(L), dtype=np.float32)
    return (loss, wf)
